# revision 51
# baseline (speedup 1.0000x reference)
"""ContactsFittingLoss on 8 Trainium2 NeuronCores (Bass/Tile).

Row-parallel with spatial candidate pruning:
  - verts are KD-partitioned (median splits) into 128-vert tiles; for each
    tile the host derives a provably-sufficient candidate set of obj points
    (per-vert probe 5-NN radius bounds + octant-bbox union test), cutting
    the N x P distance scan ~16x while keeping the top-K selection exact.
  - tiles are snake-dealt (sorted by candidate count) onto the 8 cores so
    all cores share one slot-width profile (a single SPMD program) with
    per-slot widths in 128 steps; total scanned columns ~8.6k vs 16.4k.
  - Gaussian contact weights w (anchor argmin + mahalanobis + 32-way group
    max normalization) are O(N*32) and computed host-side; the device gets
    w^2 directly, so no anchor phase and no collective is needed.
  - Per core, per slot: negated squared distances via a bf16 hi/lo split
    matmul (13-row contraction, ~fp32 accuracy) into PSUM, then one of two
    scan paths chosen to balance engines: Act drains PSUM to bf16 SBUF and
    the DVE pair-maxes (2-byte fast mode) + bf16 max8, or the DVE max8
    runs straight off PSUM (fp32 ranking, bf16 top-8 out). Mid-loop tail
    turns top-8 into sum-of-top-K, dots with w^2; a 1-col matmul against a
    ones column collapses partials so the output DMA is one descriptor.
Host sums the 8 per-core scalars into the mean.
"""
import numpy as np
import ml_dtypes
import orjson

import concourse.bass as bass
import concourse.mybir as mybir
from concourse.tile import TileContext
from concourse.bass_utils import run_bass_kernel_spmd

F32 = mybir.dt.float32
BF16 = mybir.dt.bfloat16
NA = 32
LOG_2PI = float(np.log(2.0 * np.pi))
NCORES = 8
TS = 128          # verts per tile (partition dim)

# ---------------------------------------------------------------------------
# Workaround: this container's walrus rejects instructions with >1 sync wait;
# Tile occasionally emits more. Split extras onto NoOps at serialization.
# ---------------------------------------------------------------------------
_uid = [0]


def _split_waits(d):
    for f in d.get('functions', []):
        for blk in f.get('blocks', []):
            out = []
            for ins in blk.get('instructions', []):
                si = ins.get('sync_info')
                ow = (si or {}).get('on_wait') or []
                if len(ow) > 1:
                    for w in ow[:-1]:
                        _uid[0] += 1
                        out.append({'debug': ins.get('debug', 0),
                                    'engine': ins['engine'],
                                    'ins': [], 'outs': [],
                                    'name': f"I-waitsplit-{_uid[0]}",
                                    'opcode': 'NoOp',
                                    'sync_info': {'on_update': [],
                                                  'on_wait': [w]}})
                    si['on_wait'] = ow[-1:]
                out.append(ins)
            blk['instructions'] = out
    return d


if not getattr(bass.Bass, '_cf_waitsplit', False):
    _orig_tjb = bass.Bass.to_json_bytes

    def _patched_tjb(self):
        return orjson.dumps(_split_waits(orjson.loads(_orig_tjb(self))))

    bass.Bass.to_json_bytes = _patched_tjb
    bass.Bass._cf_waitsplit = True


# ---------------------------------------------------------------------------
# Host-side prep: weights, KD tiling, candidate pruning, operand packing
# ---------------------------------------------------------------------------
def _to_bf16(x):
    return np.asarray(x, np.float32).astype(ml_dtypes.bfloat16)


def _hi_lo(x):
    h = _to_bf16(x)
    l = _to_bf16(np.asarray(x, np.float32) - h.astype(np.float32))
    return h, l


def _host_weights(V, A, cg):
    """Exact per-vert contact weight (mirrors the reference math)."""
    d2a = ((V[:, None, :] - A[None, :, :]) ** 2).sum(-1)
    aidx = np.argmin(d2a, axis=-1)
    zero_g = np.all(cg == 0.0, axis=-1)
    means = cg[:, :3] + A
    covs = cg[:, 3:].reshape(NA, 3, 3)
    covs_safe = np.where(zero_g[:, None, None], np.eye(3, dtype=np.float64),
                         covs)
    chol = np.linalg.cholesky(covs_safe)
    logdet = 2.0 * np.sum(np.log(np.diagonal(chol, axis1=-2, axis2=-1)), -1)
    inv = np.linalg.inv(covs_safe)
    diff = V - means[aidx]
    maha = np.einsum('ni,nij,nj->n', diff, inv[aidx], diff)
    logp = -0.5 * (maha + logdet[aidx] + 3.0 * LOG_2PI)
    w = np.exp(logp)
    gmax = np.zeros(NA)
    np.maximum.at(gmax, aidx, w)
    norm = np.where(gmax > 1.0, gmax, 1.0)
    w = w / norm[aidx]
    w = np.where(w > 0.01, w, 0.0)
    w = np.where(zero_g[aidx], 0.0, w)
    return w.astype(np.float32)


def _kd_perm(V, depth):
    """Balanced KD partition permutation: leaves of equal size in order."""
    out = []

    def split(ids, d):
        if d == 0:
            out.append(ids)
            return
        pts = V[ids]
        dim = int(np.argmax(pts.max(0) - pts.min(0)))
        order = np.argsort(pts[:, dim], kind='stable')
        h = len(ids) // 2
        split(ids[order[:h]], d - 1)
        split(ids[order[h:]], d - 1)

    split(np.arange(len(V)), depth)
    return np.concatenate(out)


def _candidates(tiles, Y, K, nsub=6):
    """Per-tile candidate obj-point lists guaranteed to contain every
    vert's K nearest. Bound: each vert v has K points within
    UB_v = min_probe(d(v,probe) + dK(probe)); a point can only be in
    v's top-K if it is within UB_v of v. Candidate test: union over
    per-tile octants of { d(p, octant bbox) <= max UB_v in octant }."""
    nt, TSz, _ = tiles.shape
    qs = (np.arange(nsub) + 0.5) / nsub
    g = np.quantile(tiles, qs, axis=1)            # [nsub, nt, 3]
    g = np.moveaxis(g, 0, 1)                      # [nt, nsub, 3]
    px = g[:, :, 0][:, :, None, None]
    py = g[:, :, 1][:, None, :, None]
    pz = g[:, :, 2][:, None, None, :]
    shape = (nt, nsub, nsub, nsub)
    probes = np.stack([np.broadcast_to(px, shape),
                       np.broadcast_to(py, shape),
                       np.broadcast_to(pz, shape)], -1).reshape(nt, -1, 3)
    npb = probes.shape[1]
    flat = probes.reshape(-1, 3).astype(np.float32)
    Y32 = Y.astype(np.float32)
    y2 = (Y32 ** 2).sum(-1)
    dK = np.empty(flat.shape[0], np.float32)
    step = 2048
    for i in range(0, flat.shape[0], step):
        pc = flat[i:i + step]
        d2 = ((pc ** 2).sum(-1)[:, None] + y2[None, :]
              - 2.0 * (pc @ Y32.T))
        dK[i:i + step] = np.sqrt(np.maximum(
            np.partition(d2, K - 1, axis=1)[:, K - 1], 0.0))
    dK = dK.reshape(nt, npb)
    dvp = np.sqrt(((tiles[:, :, None, :].astype(np.float32)
                    - probes[:, None, :, :].astype(np.float32)) ** 2).sum(-1))
    ubv = (dvp + dK[:, None, :]).min(-1) + 1e-5           # [nt, TS]

    # octant split (median per coord) -> per-octant bbox + UB
    med = np.median(tiles, axis=1)                        # [nt, 3]
    oct_id = ((tiles > med[:, None, :]) *
              np.array([1, 2, 4])).sum(-1)                # [nt, TS]
    t32 = tiles.astype(np.float32)
    LO = np.full((nt, 8, 3), np.inf, np.float32)
    HI = np.full((nt, 8, 3), -np.inf, np.float32)
    UBo = np.full((nt, 8), -np.inf, np.float32)
    for o in range(8):
        sel = oct_id == o                                 # [nt, TS]
        selm = np.where(sel[:, :, None], t32, np.inf)
        LO[:, o] = selm.min(1)
        HI[:, o] = np.where(sel[:, :, None], t32, -np.inf).max(1)
        UBo[:, o] = np.where(sel, ubv.astype(np.float32), -np.inf).max(1)
    mask = np.zeros((nt, Y.shape[0]), bool)
    for o in range(8):
        db2 = (np.maximum(np.maximum(LO[:, o][:, None, :] - Y32[None, :, :],
                                     Y32[None, :, :] - HI[:, o][:, None, :]),
                          0.0) ** 2).sum(-1)
        mask |= db2 <= (UBo[:, o][:, None]) ** 2
    counts = np.maximum(mask.sum(1), 8)
    widths = np.ceil(counts / 64).astype(np.int64) * 64   # 64-mult per tile
    widths = np.minimum(widths, int(np.ceil(Y.shape[0] / 64) * 64))
    C = int(widths.max())
    ids = np.zeros((nt, C), np.int64)
    pad = np.zeros((nt, C), bool)
    for t in range(nt):
        ii = np.nonzero(mask[t])[0][:C]
        ids[t, :len(ii)] = ii
        pad[t, len(ii):] = True
    return ids, pad, widths


def _pack_y(Yg):
    """[W, 3] obj pts -> [13, W] bf16 rhs rows."""
    YT = Yg.T
    y2 = (YT ** 2).sum(0)
    yh, yl = _hi_lo(YT)
    y2h, y2l = _hi_lo(y2)
    out = np.zeros((13, Yg.shape[0]), ml_dtypes.bfloat16)
    out[0:3] = yh
    out[3:6] = yl
    out[6:9] = yh
    out[9] = y2h
    out[10] = y2l
    out[11] = 1.0
    out[12] = 1.0
    return out


def _pack_v(Vc):
    """[R, 3] verts -> [13, R] bf16 lhs rows; out = 2v.y - |y|^2 - |v|^2."""
    VT = Vc.T
    v2 = (VT ** 2).sum(0)
    vh, vl = _hi_lo(2.0 * VT)
    v2h, v2l = _hi_lo(v2)
    out = np.zeros((13, Vc.shape[0]), ml_dtypes.bfloat16)
    out[0:3] = vh
    out[3:6] = vh
    out[6:9] = vl
    out[9] = -1.0
    out[10] = -1.0
    out[11] = -v2h
    out[12] = -v2l
    return out


def _host_prep(verts, anchor_verts, obj_pts, contact_gaussians, K):
    V = np.asarray(verts[0], np.float64)
    Y = np.asarray(obj_pts[0], np.float64)
    A = np.asarray(anchor_verts[0], np.float64)
    cg = np.asarray(contact_gaussians, np.float64)
    N, P = V.shape[0], Y.shape[0]

    w_all = _host_weights(V, A, cg)               # [N] float32
    depth = int(np.log2(N // TS))
    pv = _kd_perm(V.astype(np.float32), depth)    # [N]
    Vs = V[pv]
    nt = N // TS
    assert nt % NCORES == 0
    tiles = Vs.reshape(nt, TS, 3)
    w2t = (w_all[pv] ** 2).astype(np.float32).reshape(nt, TS)
    ids, pad, widths = _candidates(tiles, Y, K)

    # snake-deal tiles (sorted by width desc) into NCORES x n_slots so
    # every core shares one slot-width profile (SPMD program shape);
    # interleave ranks fat/thin so drain-pairs (2s, 2s+1) are balanced
    n_slots = nt // NCORES
    order = np.argsort(-widths, kind='stable')
    ranks = np.empty(n_slots, np.int64)
    ranks[0::2] = np.arange(n_slots // 2)
    ranks[1::2] = n_slots - 1 - np.arange(n_slots - n_slots // 2)
    slot_w_sorted = widths[order].reshape(n_slots, NCORES).max(1).astype(int)
    slot_w = slot_w_sorted[ranks]
    Wtot = int(slot_w.sum())

    cores = []
    for c in range(NCORES):
        rhsb = np.zeros((13, Wtot), ml_dtypes.bfloat16)
        Vc = np.zeros((n_slots * TS, 3))
        w2c = np.ones((TS, n_slots + 1), np.float32)
        off = 0
        for s in range(n_slots):
            t = int(order[int(ranks[s]) * NCORES + c])
            Ws = int(slot_w[s])
            Yg = Y[ids[t, :Ws]].copy()
            Yg[pad[t, :Ws]] = 1.0e3
            rhsb[:, off:off + Ws] = _pack_y(Yg)
            Vc[s * TS:(s + 1) * TS] = tiles[t]
            w2c[:, s] = w2t[t]
            off += Ws
        # one operand tensor [13, NT*TS + Wtot]: verts block then candidates
        ops = np.concatenate([_pack_v(Vc), rhsb], axis=1)
        cores.append({
            "ops": np.ascontiguousarray(ops),
            "w2": np.ascontiguousarray(w2c),
        })
    return dict(cores=cores, slot_w=tuple(int(x) for x in slot_w),
                N=N, P=P, nt=nt)


def _pack_core(prep, core):
    return prep["cores"][core]


# ---------------------------------------------------------------------------
# Device program
# ---------------------------------------------------------------------------
def _build_kernel(slot_w=(1024,) * 16, K=5, n_cores=8):
    """One 128-vert tile per slot; slot s scans slot_w[s] candidate cols.
    Slots are drained from PSUM in pairs to halve Act instruction count."""
    NT = len(slot_w)
    Wtot = int(sum(slot_w))
    L = NT * TS                           # verts block width in ops tensor
    pmW = min(2048, max(512, max(slot_w)))
    bufs = max(1, min(4, 8 // ((pmW + 511) // 512)))

    # assign each slot a scan path to balance Act vs DVE busy time:
    # 'A' = Act drains PSUM->bf16, DVE pair-maxes (fast mode) + bf16 max8
    # 'D' = DVE max8 straight from PSUM (fp32 ranking, bf16 top-8 out)
    act_t, dve_t = 0.0, 0.0
    path = [None] * NT
    for t in sorted(range(NT), key=lambda i: slot_w[i]):
        W = slot_w[t]
        a_act = (W + 86) * 0.833 + 160
        a_dve = (W // 4 + W // 2 + 58) * 1.042
        d_dve = (W + 120) * 1.042
        if max(act_t + a_act, dve_t + a_dve) <= max(act_t, dve_t + d_dve):
            path[t] = 'A'
            act_t += a_act
            dve_t += a_dve
        else:
            path[t] = 'D'
            dve_t += d_dve
    nc = bass.Bass(num_devices=n_cores)

    ops_d = nc.dram_tensor("ops", [13, L + Wtot], BF16, kind="ExternalInput")
    w2_d = nc.dram_tensor("w2", [TS, NT + 1], F32, kind="ExternalInput")
    part_d = nc.dram_tensor("part", [1], F32, kind="ExternalOutput")

    offs = [L]
    for w in slot_w:
        offs.append(offs[-1] + w)

    with TileContext(nc) as tc:
        with tc.tile_pool(name="const", bufs=1) as cp:
            ops = cp.tile([13, L + Wtot], BF16, tag="ops")
            w2 = cp.tile([TS, NT + 1], F32, tag="w2")
            WK = cp.tile([TS, NT * 8], BF16, tag="WK")

            # preload the activation table first on the Act queue, then
            # HWDGE issues (SP + Act); first chunk = verts + slots 0-1
            warm = cp.tile([1, 2], F32, tag="warm")
            nc.gpsimd.memset(warm[:, 0:1], 0.0)
            nc.scalar.copy(warm[:, 1:2], warm[:, 0:1])
            cut1 = offs[min(2, NT)]
            cut2 = offs[min(8, NT)]
            nc.sync.dma_start(ops[:, 0:cut1], ops_d[:, 0:cut1])
            nc.scalar.dma_start(ops[:, cut1:cut2], ops_d[:, cut1:cut2])
            nc.sync.dma_start(ops[:, cut2:], ops_d[:, cut2:])
            nc.scalar.dma_start(w2[:], w2_d[:])

            # warm the PE p-state during the DMA shadow: ~5us of dummy
            # back-to-back matmuls (clock ramps after ~3us continuous)
            dmy = cp.tile([13, 512], BF16, tag="dmy")
            nc.gpsimd.memset(dmy[:], 0.0)
            with tc.tile_pool(name="psD", bufs=1, space="PSUM") as psD:
                pd = psD.tile([TS, 512], F32, tag="pd")
                for _ in range(12):
                    nc.tensor.matmul(pd[:], dmy[:, 0:TS], dmy[:])

            tail_cuts = {NT // 2, NT - 2} if NT >= 8 else set()
            tail_prts = []
            tail_lo = [0]

            with tc.tile_pool(name="tail", bufs=1) as tl:
              with tc.tile_pool(name="psM", bufs=bufs, space="PSUM") as psM, \
                   tc.tile_pool(name="cand", bufs=3) as cnd:
                for t in range(NT):
                    W = slot_w[t]
                    base = offs[t]
                    h1 = W // 2
                    if W <= 2048:
                        pm = psM.tile([TS, pmW], F32, tag="pm")
                        for off in range(0, W, 512):
                            qw = min(512, W - off)
                            nc.tensor.matmul(
                                pm[:, off:off + qw],
                                ops[:, t * TS:(t + 1) * TS],
                                ops[:, base + off:base + off + qw])
                        if path[t] == 'A':
                            sb = cnd.tile([TS, W], BF16, tag=f"sb{W}")
                            nc.scalar.copy(sb[:], pm[:, 0:W])
                            mx = cnd.tile([TS, h1], BF16, tag=f"mx{W}")
                            nc.vector.tensor_tensor(mx[:], sb[:, 0:h1],
                                                    sb[:, h1:W],
                                                    op=mybir.AluOpType.max)
                            nc.vector.max(out=WK[:, t * 8:(t + 1) * 8],
                                          in_=mx[:])
                        else:
                            nc.vector.max(out=WK[:, t * 8:(t + 1) * 8],
                                          in_=pm[:, 0:W])
                    else:
                        nch = (W + 2047) // 2048
                        cands = cnd.tile([TS, nch * 8], F32, tag="cands")
                        for c in range(nch):
                            cw = min(2048, W - c * 2048)
                            pm = psM.tile([TS, pmW], F32, tag="pm")
                            for off in range(0, cw, 512):
                                qw = min(512, cw - off)
                                o2 = base + c * 2048 + off
                                nc.tensor.matmul(
                                    pm[:, off:off + qw],
                                    ops[:, t * TS:(t + 1) * TS],
                                    ops[:, o2:o2 + qw])
                            nc.vector.max(out=cands[:, c * 8:(c + 1) * 8],
                                          in_=pm[:, 0:cw])
                        t8 = cnd.tile([TS, 8], F32, tag="t8")
                        nc.vector.max(out=t8[:], in_=cands[:])
                        nc.vector.tensor_copy(WK[:, t * 8:(t + 1) * 8],
                                              t8[:])
                    if t + 1 in tail_cuts:
                        # mid-loop partial tail over completed slots
                        tail_prts.append(_emit_tail(nc, tl, WK, w2,
                                                    tail_lo[0], t + 1, NT, K,
                                                    f"p{t + 1}"))
                        tail_lo[0] = t + 1

              with tc.tile_pool(name="psT", bufs=1, space="PSUM") as psT:
                prtB = _emit_tail(nc, tl, WK, w2, tail_lo[0], NT, NT, K, "B")
                for p in tail_prts:
                    nc.vector.tensor_add(prtB[:], prtB[:], p[:])
                # cross-partition total via 1-col matmul against the
                # ones column -> single-descriptor output DMA
                tot = psT.tile([1, 1], F32, tag="tot")
                nc.tensor.matmul(tot[:], w2[:, NT:NT + 1], prtB[:])
                res = tl.tile([1, 1], F32, tag="res")
                nc.vector.tensor_copy(res[:], tot[:])
                nc.sync.dma_start(part_d[:], res[:, 0])
    return nc


def _emit_tail(nc, tl, WK, w2, lo, hi, NT, K, tag):
    """knn2 = max(-top8, 0) for slots [lo,hi); dot with w2; row partials."""
    n = hi - lo
    knn2 = tl.tile([TS, n * 8], F32, tag=f"knn2{tag}")
    nc.vector.tensor_scalar(knn2[:], WK[:, lo * 8:hi * 8], -1.0, 0.0,
                            op0=mybir.AluOpType.mult,
                            op1=mybir.AluOpType.max)
    wfin = tl.tile([TS, n * K], F32, tag=f"wfin{tag}")
    k3 = knn2[:].rearrange("p (t k) -> p t k", t=n, k=8)
    w3 = wfin[:].rearrange("p (t k) -> p t k", t=n, k=K)
    w2b = w2[:, lo:hi].unsqueeze(2).to_broadcast([TS, n, K])
    nc.vector.tensor_mul(w3, k3[:, :, :K], w2b)
    prt = tl.tile([TS, 1], F32, tag=f"prt{tag}")
    nc.vector.reduce_sum(prt[:], wfin[:], axis=mybir.AxisListType.X)
    return prt


_NC_CACHE = {}


def kernel(**inputs) -> np.ndarray:
    verts = np.asarray(inputs["verts"], np.float32)
    anchor_verts = np.asarray(inputs["anchor_verts"], np.float32)
    obj_pts = np.asarray(inputs["obj_pts"], np.float32)
    cg = np.asarray(inputs["contact_gaussians"], np.float32)
    K = int(np.asarray(inputs["K"]))
    B, N, _ = verts.shape
    P = obj_pts.shape[1]
    assert B == 1 and 1 <= K <= 8

    prep = _host_prep(verts, anchor_verts, obj_pts, cg, K)
    in_maps = [_pack_core(prep, c) for c in range(NCORES)]

    key = (prep["slot_w"], K)
    if key not in _NC_CACHE:
        _NC_CACHE[key] = _build_kernel(slot_w=prep["slot_w"], K=K,
                                       n_cores=NCORES)
    nc = _NC_CACHE[key]
    res = run_bass_kernel_spmd(nc, in_maps, core_ids=list(range(NCORES)))

    total = np.float32(0.0)
    for c in range(NCORES):
        total += np.float32(res.results[c]["part"].reshape(-1)[0])
    return np.float32(total / np.float32(N * K))


# revision 53
# speedup vs baseline: 1.0482x; 1.0482x over previous
"""ContactsFittingLoss on 8 Trainium2 NeuronCores (Bass/Tile).

Row-parallel with spatial candidate pruning:
  - verts are KD-partitioned (median splits) into 128-vert tiles; for each
    tile the host derives a provably-sufficient candidate set of obj points
    (per-vert probe 5-NN radius bounds + octant-bbox union test), cutting
    the N x P distance scan ~16x while keeping the top-K selection exact.
  - tiles are snake-dealt (sorted by candidate count) onto the 8 cores so
    all cores share one slot-width profile (a single SPMD program) with
    per-slot widths in 128 steps; total scanned columns ~8.6k vs 16.4k.
  - Gaussian contact weights w (anchor argmin + mahalanobis + 32-way group
    max normalization) are O(N*32) and computed host-side; the device gets
    w^2 directly, so no anchor phase and no collective is needed.
  - Per core, per slot: negated squared distances via a bf16 hi/lo split
    matmul (13-row contraction, ~fp32 accuracy) into PSUM, then one of two
    scan paths chosen to balance engines: Act drains PSUM to bf16 SBUF and
    the DVE pair-maxes (2-byte fast mode) + bf16 max8, or the DVE max8
    runs straight off PSUM (fp32 ranking, bf16 top-8 out). Mid-loop tail
    turns top-8 into sum-of-top-K, dots with w^2; a 1-col matmul against a
    ones column collapses partials so the output DMA is one descriptor.
Host sums the 8 per-core scalars into the mean.
"""
import numpy as np
import ml_dtypes
import orjson

import concourse.bass as bass
import concourse.mybir as mybir
from concourse.tile import TileContext
from concourse.bass_utils import run_bass_kernel_spmd

F32 = mybir.dt.float32
BF16 = mybir.dt.bfloat16
NA = 32
LOG_2PI = float(np.log(2.0 * np.pi))
NCORES = 8
TS = 128          # verts per tile (partition dim)

# ---------------------------------------------------------------------------
# Workaround: this container's walrus rejects instructions with >1 sync wait;
# Tile occasionally emits more. Split extras onto NoOps at serialization.
# ---------------------------------------------------------------------------
_uid = [0]


def _split_waits(d):
    for f in d.get('functions', []):
        for blk in f.get('blocks', []):
            out = []
            for ins in blk.get('instructions', []):
                si = ins.get('sync_info')
                ow = (si or {}).get('on_wait') or []
                if len(ow) > 1:
                    for w in ow[:-1]:
                        _uid[0] += 1
                        out.append({'debug': ins.get('debug', 0),
                                    'engine': ins['engine'],
                                    'ins': [], 'outs': [],
                                    'name': f"I-waitsplit-{_uid[0]}",
                                    'opcode': 'NoOp',
                                    'sync_info': {'on_update': [],
                                                  'on_wait': [w]}})
                    si['on_wait'] = ow[-1:]
                out.append(ins)
            blk['instructions'] = out
    return d


if not getattr(bass.Bass, '_cf_waitsplit', False):
    _orig_tjb = bass.Bass.to_json_bytes

    def _patched_tjb(self):
        return orjson.dumps(_split_waits(orjson.loads(_orig_tjb(self))))

    bass.Bass.to_json_bytes = _patched_tjb
    bass.Bass._cf_waitsplit = True


# ---------------------------------------------------------------------------
# Host-side prep: weights, KD tiling, candidate pruning, operand packing
# ---------------------------------------------------------------------------
def _to_bf16(x):
    return np.asarray(x, np.float32).astype(ml_dtypes.bfloat16)


def _hi_lo(x):
    h = _to_bf16(x)
    l = _to_bf16(np.asarray(x, np.float32) - h.astype(np.float32))
    return h, l


def _host_weights(V, A, cg):
    """Exact per-vert contact weight (mirrors the reference math)."""
    d2a = ((V[:, None, :] - A[None, :, :]) ** 2).sum(-1)
    aidx = np.argmin(d2a, axis=-1)
    zero_g = np.all(cg == 0.0, axis=-1)
    means = cg[:, :3] + A
    covs = cg[:, 3:].reshape(NA, 3, 3)
    covs_safe = np.where(zero_g[:, None, None], np.eye(3, dtype=np.float64),
                         covs)
    chol = np.linalg.cholesky(covs_safe)
    logdet = 2.0 * np.sum(np.log(np.diagonal(chol, axis1=-2, axis2=-1)), -1)
    inv = np.linalg.inv(covs_safe)
    diff = V - means[aidx]
    maha = np.einsum('ni,nij,nj->n', diff, inv[aidx], diff)
    logp = -0.5 * (maha + logdet[aidx] + 3.0 * LOG_2PI)
    w = np.exp(logp)
    gmax = np.zeros(NA)
    np.maximum.at(gmax, aidx, w)
    norm = np.where(gmax > 1.0, gmax, 1.0)
    w = w / norm[aidx]
    w = np.where(w > 0.01, w, 0.0)
    w = np.where(zero_g[aidx], 0.0, w)
    return w.astype(np.float32)


def _kd_perm(V, depth):
    """Balanced KD partition permutation: leaves of equal size in order."""
    out = []

    def split(ids, d):
        if d == 0:
            out.append(ids)
            return
        pts = V[ids]
        dim = int(np.argmax(pts.max(0) - pts.min(0)))
        order = np.argsort(pts[:, dim], kind='stable')
        h = len(ids) // 2
        split(ids[order[:h]], d - 1)
        split(ids[order[h:]], d - 1)

    split(np.arange(len(V)), depth)
    return np.concatenate(out)


def _candidates(tiles, Y, K, nsub=6):
    """Per-tile candidate obj-point lists guaranteed to contain every
    vert's K nearest. Bound: each vert v has K points within
    UB_v = min_probe(d(v,probe) + dK(probe)); a point can only be in
    v's top-K if it is within UB_v of v. Candidate test: union over
    per-tile octants of { d(p, octant bbox) <= max UB_v in octant }."""
    nt, TSz, _ = tiles.shape
    qs = (np.arange(nsub) + 0.5) / nsub
    g = np.quantile(tiles, qs, axis=1)            # [nsub, nt, 3]
    g = np.moveaxis(g, 0, 1)                      # [nt, nsub, 3]
    px = g[:, :, 0][:, :, None, None]
    py = g[:, :, 1][:, None, :, None]
    pz = g[:, :, 2][:, None, None, :]
    shape = (nt, nsub, nsub, nsub)
    probes = np.stack([np.broadcast_to(px, shape),
                       np.broadcast_to(py, shape),
                       np.broadcast_to(pz, shape)], -1).reshape(nt, -1, 3)
    npb = probes.shape[1]
    flat = probes.reshape(-1, 3).astype(np.float32)
    Y32 = Y.astype(np.float32)
    y2 = (Y32 ** 2).sum(-1)
    dK = np.empty(flat.shape[0], np.float32)
    step = 2048
    for i in range(0, flat.shape[0], step):
        pc = flat[i:i + step]
        d2 = ((pc ** 2).sum(-1)[:, None] + y2[None, :]
              - 2.0 * (pc @ Y32.T))
        dK[i:i + step] = np.sqrt(np.maximum(
            np.partition(d2, K - 1, axis=1)[:, K - 1], 0.0))
    dK = dK.reshape(nt, npb)
    dvp = np.sqrt(((tiles[:, :, None, :].astype(np.float32)
                    - probes[:, None, :, :].astype(np.float32)) ** 2).sum(-1))
    ubv = (dvp + dK[:, None, :]).min(-1) + 1e-5           # [nt, TS]

    # octant split (median per coord) -> per-octant bbox + UB
    med = np.median(tiles, axis=1)                        # [nt, 3]
    oct_id = ((tiles > med[:, None, :]) *
              np.array([1, 2, 4])).sum(-1)                # [nt, TS]
    t32 = tiles.astype(np.float32)
    LO = np.full((nt, 8, 3), np.inf, np.float32)
    HI = np.full((nt, 8, 3), -np.inf, np.float32)
    UBo = np.full((nt, 8), -np.inf, np.float32)
    for o in range(8):
        sel = oct_id == o                                 # [nt, TS]
        selm = np.where(sel[:, :, None], t32, np.inf)
        LO[:, o] = selm.min(1)
        HI[:, o] = np.where(sel[:, :, None], t32, -np.inf).max(1)
        UBo[:, o] = np.where(sel, ubv.astype(np.float32), -np.inf).max(1)
    mask = np.zeros((nt, Y.shape[0]), bool)
    for o in range(8):
        db2 = (np.maximum(np.maximum(LO[:, o][:, None, :] - Y32[None, :, :],
                                     Y32[None, :, :] - HI[:, o][:, None, :]),
                          0.0) ** 2).sum(-1)
        mask |= db2 <= (UBo[:, o][:, None]) ** 2
    counts = np.maximum(mask.sum(1), 8)
    widths = np.ceil(counts / 64).astype(np.int64) * 64   # 64-mult per tile
    widths = np.minimum(widths, int(np.ceil(Y.shape[0] / 64) * 64))
    C = int(widths.max())
    ids = np.zeros((nt, C), np.int64)
    pad = np.zeros((nt, C), bool)
    for t in range(nt):
        ii = np.nonzero(mask[t])[0][:C]
        ids[t, :len(ii)] = ii
        pad[t, len(ii):] = True
    return ids, pad, widths


def _pack_y(Yg):
    """[W, 3] obj pts -> [13, W] bf16 rhs rows."""
    YT = Yg.T
    y2 = (YT ** 2).sum(0)
    yh, yl = _hi_lo(YT)
    y2h, y2l = _hi_lo(y2)
    out = np.zeros((13, Yg.shape[0]), ml_dtypes.bfloat16)
    out[0:3] = yh
    out[3:6] = yl
    out[6:9] = yh
    out[9] = y2h
    out[10] = y2l
    out[11] = 1.0
    out[12] = 1.0
    return out


def _pack_v(Vc):
    """[R, 3] verts -> [13, R] bf16 lhs rows; out = 2v.y - |y|^2 - |v|^2."""
    VT = Vc.T
    v2 = (VT ** 2).sum(0)
    vh, vl = _hi_lo(2.0 * VT)
    v2h, v2l = _hi_lo(v2)
    out = np.zeros((13, Vc.shape[0]), ml_dtypes.bfloat16)
    out[0:3] = vh
    out[3:6] = vh
    out[6:9] = vl
    out[9] = -1.0
    out[10] = -1.0
    out[11] = -v2h
    out[12] = -v2l
    return out


def _host_prep(verts, anchor_verts, obj_pts, contact_gaussians, K):
    V = np.asarray(verts[0], np.float64)
    Y = np.asarray(obj_pts[0], np.float64)
    A = np.asarray(anchor_verts[0], np.float64)
    cg = np.asarray(contact_gaussians, np.float64)
    N, P = V.shape[0], Y.shape[0]

    w_all = _host_weights(V, A, cg)               # [N] float32
    depth = int(np.log2(N // TS))
    pv = _kd_perm(V.astype(np.float32), depth)    # [N]
    Vs = V[pv]
    nt = N // TS
    assert nt % NCORES == 0
    tiles = Vs.reshape(nt, TS, 3)
    w2t = (w_all[pv] ** 2).astype(np.float32).reshape(nt, TS)
    ids, pad, widths = _candidates(tiles, Y, K)

    # snake-deal tiles (sorted by width desc) into NCORES x n_slots so
    # every core shares one slot-width profile (SPMD program shape);
    # interleave ranks fat/thin so drain-pairs (2s, 2s+1) are balanced
    n_slots = nt // NCORES
    order = np.argsort(-widths, kind='stable')
    ranks = np.empty(n_slots, np.int64)
    ranks[0::2] = np.arange(n_slots // 2)
    ranks[1::2] = n_slots - 1 - np.arange(n_slots - n_slots // 2)
    slot_w_sorted = widths[order].reshape(n_slots, NCORES).max(1).astype(int)
    slot_w = slot_w_sorted[ranks]
    Wtot = int(slot_w.sum())

    cores = []
    for c in range(NCORES):
        rhsb = np.zeros((13, Wtot), ml_dtypes.bfloat16)
        Vc = np.zeros((n_slots * TS, 3))
        w2c = np.ones((TS, n_slots + 1), np.float32)
        off = 0
        for s in range(n_slots):
            t = int(order[int(ranks[s]) * NCORES + c])
            Ws = int(slot_w[s])
            Yg = Y[ids[t, :Ws]].copy()
            Yg[pad[t, :Ws]] = 1.0e3
            rhsb[:, off:off + Ws] = _pack_y(Yg)
            Vc[s * TS:(s + 1) * TS] = tiles[t]
            w2c[:, s] = w2t[t]
            off += Ws
        # one operand tensor [13, NT*TS + Wtot]: verts block then candidates
        ops = np.concatenate([_pack_v(Vc), rhsb], axis=1)
        cores.append({
            "ops": np.ascontiguousarray(ops),
            "w2": np.ascontiguousarray(w2c),
        })
    return dict(cores=cores, slot_w=tuple(int(x) for x in slot_w),
                N=N, P=P, nt=nt)


def _pack_core(prep, core):
    return prep["cores"][core]


# ---------------------------------------------------------------------------
# Device program
# ---------------------------------------------------------------------------
def _build_kernel(slot_w=(1024,) * 16, K=5, n_cores=8):
    """One 128-vert tile per slot; slot s scans slot_w[s] candidate cols.
    Slots are drained from PSUM in pairs to halve Act instruction count."""
    NT = len(slot_w)
    Wtot = int(sum(slot_w))
    L = NT * TS                           # verts block width in ops tensor
    pmW = min(2048, max(512, max(slot_w)))
    bufs = max(1, min(4, 8 // ((pmW + 511) // 512)))

    # assign each slot a scan path to balance Act vs DVE busy time:
    # 'A' = Act drains PSUM->bf16, DVE pair-maxes (fast mode) + bf16 max8
    # 'D' = DVE max8 straight from PSUM (fp32 ranking, bf16 top-8 out)
    act_t, dve_t = 0.0, 0.0
    path = [None] * NT
    for t in sorted(range(NT), key=lambda i: slot_w[i]):
        W = slot_w[t]
        a_act = (W + 86) * 0.833 + 160
        a_dve = (W // 4 + W // 2 + 58) * 1.042
        d_dve = (W + 120) * 1.042
        if max(act_t + a_act, dve_t + a_dve) <= max(act_t, dve_t + d_dve):
            path[t] = 'A'
            act_t += a_act
            dve_t += a_dve
        else:
            path[t] = 'D'
            dve_t += d_dve
    nc = bass.Bass(num_devices=n_cores)

    ops_d = nc.dram_tensor("ops", [13, L + Wtot], BF16, kind="ExternalInput")
    w2_d = nc.dram_tensor("w2", [TS, NT + 1], F32, kind="ExternalInput")
    part_d = nc.dram_tensor("part", [1], F32, kind="ExternalOutput")

    offs = [L]
    for w in slot_w:
        offs.append(offs[-1] + w)

    with TileContext(nc) as tc:
        with tc.tile_pool(name="const", bufs=1) as cp:
            ops = cp.tile([13, L + Wtot], BF16, tag="ops")
            w2 = cp.tile([TS, NT + 1], F32, tag="w2")
            WK = cp.tile([TS, NT * 8], BF16, tag="WK")

            # preload the activation table first on the Act queue, then
            # HWDGE issues (SP + Act); first chunk = verts + slots 0-1
            warm = cp.tile([1, 2], F32, tag="warm")
            nc.gpsimd.memset(warm[:, 0:1], 0.0)
            nc.scalar.copy(warm[:, 1:2], warm[:, 0:1])
            cut1 = offs[min(2, NT)]
            cut2 = offs[min(8, NT)]
            nc.sync.dma_start(ops[:, 0:cut1], ops_d[:, 0:cut1])
            nc.scalar.dma_start(ops[:, cut1:cut2], ops_d[:, cut1:cut2])
            nc.sync.dma_start(ops[:, cut2:], ops_d[:, cut2:])
            nc.scalar.dma_start(w2[:], w2_d[:])

            # warm the PE p-state during the DMA shadow: ~3.4us of truly
            # back-to-back dummy matmuls (alternating PSUM buffers so no
            # WAW serialization; clock ramps after ~3us continuous busy)
            dmy = cp.tile([13, 512], BF16, tag="dmy")
            nc.gpsimd.memset(dmy[:], 0.0)
            with tc.tile_pool(name="psD", bufs=2, space="PSUM") as psD:
                for _ in range(8):
                    pd = psD.tile([TS, 512], F32, tag="pd")
                    nc.tensor.matmul(pd[:], dmy[:, 0:TS], dmy[:])

            tail_cuts = {NT // 2, NT - 2} if NT >= 8 else set()
            tail_prts = []
            tail_lo = [0]

            with tc.tile_pool(name="tail", bufs=1) as tl:
              with tc.tile_pool(name="psM", bufs=bufs, space="PSUM") as psM, \
                   tc.tile_pool(name="cand", bufs=3) as cnd:
                for t in range(NT):
                    W = slot_w[t]
                    base = offs[t]
                    h1 = W // 2
                    if W <= 2048:
                        pm = psM.tile([TS, pmW], F32, tag="pm")
                        for off in range(0, W, 512):
                            qw = min(512, W - off)
                            nc.tensor.matmul(
                                pm[:, off:off + qw],
                                ops[:, t * TS:(t + 1) * TS],
                                ops[:, base + off:base + off + qw])
                        if path[t] == 'A':
                            sb = cnd.tile([TS, W], BF16, tag=f"sb{W}")
                            nc.scalar.copy(sb[:], pm[:, 0:W])
                            mx = cnd.tile([TS, h1], BF16, tag=f"mx{W}")
                            nc.vector.tensor_tensor(mx[:], sb[:, 0:h1],
                                                    sb[:, h1:W],
                                                    op=mybir.AluOpType.max)
                            nc.vector.max(out=WK[:, t * 8:(t + 1) * 8],
                                          in_=mx[:])
                        else:
                            nc.vector.max(out=WK[:, t * 8:(t + 1) * 8],
                                          in_=pm[:, 0:W])
                    else:
                        nch = (W + 2047) // 2048
                        cands = cnd.tile([TS, nch * 8], F32, tag="cands")
                        for c in range(nch):
                            cw = min(2048, W - c * 2048)
                            pm = psM.tile([TS, pmW], F32, tag="pm")
                            for off in range(0, cw, 512):
                                qw = min(512, cw - off)
                                o2 = base + c * 2048 + off
                                nc.tensor.matmul(
                                    pm[:, off:off + qw],
                                    ops[:, t * TS:(t + 1) * TS],
                                    ops[:, o2:o2 + qw])
                            nc.vector.max(out=cands[:, c * 8:(c + 1) * 8],
                                          in_=pm[:, 0:cw])
                        t8 = cnd.tile([TS, 8], F32, tag="t8")
                        nc.vector.max(out=t8[:], in_=cands[:])
                        nc.vector.tensor_copy(WK[:, t * 8:(t + 1) * 8],
                                              t8[:])
                    if t + 1 in tail_cuts:
                        # mid-loop partial tail over completed slots
                        tail_prts.append(_emit_tail(nc, tl, WK, w2,
                                                    tail_lo[0], t + 1, NT, K,
                                                    f"p{t + 1}"))
                        tail_lo[0] = t + 1

              with tc.tile_pool(name="psT", bufs=1, space="PSUM") as psT:
                prtB = _emit_tail(nc, tl, WK, w2, tail_lo[0], NT, NT, K, "B")
                for p in tail_prts:
                    nc.vector.tensor_add(prtB[:], prtB[:], p[:])
                # cross-partition total via 1-col matmul against the
                # ones column -> single-descriptor output DMA
                tot = psT.tile([1, 1], F32, tag="tot")
                nc.tensor.matmul(tot[:], w2[:, NT:NT + 1], prtB[:])
                res = tl.tile([1, 1], F32, tag="res")
                nc.vector.tensor_copy(res[:], tot[:])
                nc.sync.dma_start(part_d[:], res[:, 0])
    return nc


def _emit_tail(nc, tl, WK, w2, lo, hi, NT, K, tag):
    """knn2 = max(-top8, 0) for slots [lo,hi); dot with w2; row partials."""
    n = hi - lo
    knn2 = tl.tile([TS, n * 8], F32, tag=f"knn2{tag}")
    nc.vector.tensor_scalar(knn2[:], WK[:, lo * 8:hi * 8], -1.0, 0.0,
                            op0=mybir.AluOpType.mult,
                            op1=mybir.AluOpType.max)
    wfin = tl.tile([TS, n * K], F32, tag=f"wfin{tag}")
    k3 = knn2[:].rearrange("p (t k) -> p t k", t=n, k=8)
    w3 = wfin[:].rearrange("p (t k) -> p t k", t=n, k=K)
    w2b = w2[:, lo:hi].unsqueeze(2).to_broadcast([TS, n, K])
    nc.vector.tensor_mul(w3, k3[:, :, :K], w2b)
    prt = tl.tile([TS, 1], F32, tag=f"prt{tag}")
    nc.vector.reduce_sum(prt[:], wfin[:], axis=mybir.AxisListType.X)
    return prt


_NC_CACHE = {}


def kernel(**inputs) -> np.ndarray:
    verts = np.asarray(inputs["verts"], np.float32)
    anchor_verts = np.asarray(inputs["anchor_verts"], np.float32)
    obj_pts = np.asarray(inputs["obj_pts"], np.float32)
    cg = np.asarray(inputs["contact_gaussians"], np.float32)
    K = int(np.asarray(inputs["K"]))
    B, N, _ = verts.shape
    P = obj_pts.shape[1]
    assert B == 1 and 1 <= K <= 8

    prep = _host_prep(verts, anchor_verts, obj_pts, cg, K)
    in_maps = [_pack_core(prep, c) for c in range(NCORES)]

    key = (prep["slot_w"], K)
    if key not in _NC_CACHE:
        _NC_CACHE[key] = _build_kernel(slot_w=prep["slot_w"], K=K,
                                       n_cores=NCORES)
    nc = _NC_CACHE[key]
    res = run_bass_kernel_spmd(nc, in_maps, core_ids=list(range(NCORES)))

    total = np.float32(0.0)
    for c in range(NCORES):
        total += np.float32(res.results[c]["part"].reshape(-1)[0])
    return np.float32(total / np.float32(N * K))


# revision 54
# speedup vs baseline: 1.0716x; 1.0223x over previous
"""ContactsFittingLoss on 8 Trainium2 NeuronCores (Bass/Tile).

Row-parallel with spatial candidate pruning:
  - verts are KD-partitioned (median splits) into 128-vert tiles; for each
    tile the host derives a provably-sufficient candidate set of obj points
    (per-vert probe 5-NN radius bounds + octant-bbox union test), cutting
    the N x P distance scan ~16x while keeping the top-K selection exact.
  - tiles are snake-dealt (sorted by candidate count) onto the 8 cores so
    all cores share one slot-width profile (a single SPMD program) with
    per-slot widths in 128 steps; total scanned columns ~8.6k vs 16.4k.
  - Gaussian contact weights w (anchor argmin + mahalanobis + 32-way group
    max normalization) are O(N*32) and computed host-side; the device gets
    w^2 directly, so no anchor phase and no collective is needed.
  - Per core, per slot: negated squared distances via a bf16 hi/lo split
    matmul (13-row contraction, ~fp32 accuracy) into PSUM, then one of two
    scan paths chosen to balance engines: Act drains PSUM to bf16 SBUF and
    the DVE pair-maxes (2-byte fast mode) + bf16 max8, or the DVE max8
    runs straight off PSUM (fp32 ranking, bf16 top-8 out). Mid-loop tail
    turns top-8 into sum-of-top-K, dots with w^2; a 1-col matmul against a
    ones column collapses partials so the output DMA is one descriptor.
Host sums the 8 per-core scalars into the mean.
"""
import numpy as np
import ml_dtypes
import orjson

import concourse.bass as bass
import concourse.mybir as mybir
from concourse.tile import TileContext
from concourse.bass_utils import run_bass_kernel_spmd

F32 = mybir.dt.float32
BF16 = mybir.dt.bfloat16
NA = 32
LOG_2PI = float(np.log(2.0 * np.pi))
NCORES = 8
TS = 128          # verts per tile (partition dim)

# ---------------------------------------------------------------------------
# Workaround: this container's walrus rejects instructions with >1 sync wait;
# Tile occasionally emits more. Split extras onto NoOps at serialization.
# ---------------------------------------------------------------------------
_uid = [0]


def _split_waits(d):
    for f in d.get('functions', []):
        for blk in f.get('blocks', []):
            out = []
            for ins in blk.get('instructions', []):
                si = ins.get('sync_info')
                ow = (si or {}).get('on_wait') or []
                if len(ow) > 1:
                    for w in ow[:-1]:
                        _uid[0] += 1
                        out.append({'debug': ins.get('debug', 0),
                                    'engine': ins['engine'],
                                    'ins': [], 'outs': [],
                                    'name': f"I-waitsplit-{_uid[0]}",
                                    'opcode': 'NoOp',
                                    'sync_info': {'on_update': [],
                                                  'on_wait': [w]}})
                    si['on_wait'] = ow[-1:]
                out.append(ins)
            blk['instructions'] = out
    return d


if not getattr(bass.Bass, '_cf_waitsplit', False):
    _orig_tjb = bass.Bass.to_json_bytes

    def _patched_tjb(self):
        return orjson.dumps(_split_waits(orjson.loads(_orig_tjb(self))))

    bass.Bass.to_json_bytes = _patched_tjb
    bass.Bass._cf_waitsplit = True


# ---------------------------------------------------------------------------
# Host-side prep: weights, KD tiling, candidate pruning, operand packing
# ---------------------------------------------------------------------------
def _to_bf16(x):
    return np.asarray(x, np.float32).astype(ml_dtypes.bfloat16)


def _hi_lo(x):
    h = _to_bf16(x)
    l = _to_bf16(np.asarray(x, np.float32) - h.astype(np.float32))
    return h, l


def _host_weights(V, A, cg):
    """Exact per-vert contact weight (mirrors the reference math)."""
    d2a = ((V[:, None, :] - A[None, :, :]) ** 2).sum(-1)
    aidx = np.argmin(d2a, axis=-1)
    zero_g = np.all(cg == 0.0, axis=-1)
    means = cg[:, :3] + A
    covs = cg[:, 3:].reshape(NA, 3, 3)
    covs_safe = np.where(zero_g[:, None, None], np.eye(3, dtype=np.float64),
                         covs)
    chol = np.linalg.cholesky(covs_safe)
    logdet = 2.0 * np.sum(np.log(np.diagonal(chol, axis1=-2, axis2=-1)), -1)
    inv = np.linalg.inv(covs_safe)
    diff = V - means[aidx]
    maha = np.einsum('ni,nij,nj->n', diff, inv[aidx], diff)
    logp = -0.5 * (maha + logdet[aidx] + 3.0 * LOG_2PI)
    w = np.exp(logp)
    gmax = np.zeros(NA)
    np.maximum.at(gmax, aidx, w)
    norm = np.where(gmax > 1.0, gmax, 1.0)
    w = w / norm[aidx]
    w = np.where(w > 0.01, w, 0.0)
    w = np.where(zero_g[aidx], 0.0, w)
    return w.astype(np.float32)


def _kd_perm(V, depth):
    """Balanced KD partition permutation: leaves of equal size in order."""
    out = []

    def split(ids, d):
        if d == 0:
            out.append(ids)
            return
        pts = V[ids]
        dim = int(np.argmax(pts.max(0) - pts.min(0)))
        order = np.argsort(pts[:, dim], kind='stable')
        h = len(ids) // 2
        split(ids[order[:h]], d - 1)
        split(ids[order[h:]], d - 1)

    split(np.arange(len(V)), depth)
    return np.concatenate(out)


def _candidates(tiles, Y, K, nsub=6):
    """Per-tile candidate obj-point lists guaranteed to contain every
    vert's K nearest. Bound: each vert v has K points within
    UB_v = min_probe(d(v,probe) + dK(probe)); a point can only be in
    v's top-K if it is within UB_v of v. Candidate test: union over
    per-tile octants of { d(p, octant bbox) <= max UB_v in octant }."""
    nt, TSz, _ = tiles.shape
    qs = (np.arange(nsub) + 0.5) / nsub
    g = np.quantile(tiles, qs, axis=1)            # [nsub, nt, 3]
    g = np.moveaxis(g, 0, 1)                      # [nt, nsub, 3]
    px = g[:, :, 0][:, :, None, None]
    py = g[:, :, 1][:, None, :, None]
    pz = g[:, :, 2][:, None, None, :]
    shape = (nt, nsub, nsub, nsub)
    probes = np.stack([np.broadcast_to(px, shape),
                       np.broadcast_to(py, shape),
                       np.broadcast_to(pz, shape)], -1).reshape(nt, -1, 3)
    npb = probes.shape[1]
    flat = probes.reshape(-1, 3).astype(np.float32)
    Y32 = Y.astype(np.float32)
    y2 = (Y32 ** 2).sum(-1)
    dK = np.empty(flat.shape[0], np.float32)
    step = 2048
    for i in range(0, flat.shape[0], step):
        pc = flat[i:i + step]
        d2 = ((pc ** 2).sum(-1)[:, None] + y2[None, :]
              - 2.0 * (pc @ Y32.T))
        dK[i:i + step] = np.sqrt(np.maximum(
            np.partition(d2, K - 1, axis=1)[:, K - 1], 0.0))
    dK = dK.reshape(nt, npb)
    dvp = np.sqrt(((tiles[:, :, None, :].astype(np.float32)
                    - probes[:, None, :, :].astype(np.float32)) ** 2).sum(-1))
    ubv = (dvp + dK[:, None, :]).min(-1) + 1e-5           # [nt, TS]

    # octant split (median per coord) -> per-octant bbox + UB
    med = np.median(tiles, axis=1)                        # [nt, 3]
    oct_id = ((tiles > med[:, None, :]) *
              np.array([1, 2, 4])).sum(-1)                # [nt, TS]
    t32 = tiles.astype(np.float32)
    LO = np.full((nt, 8, 3), np.inf, np.float32)
    HI = np.full((nt, 8, 3), -np.inf, np.float32)
    UBo = np.full((nt, 8), -np.inf, np.float32)
    for o in range(8):
        sel = oct_id == o                                 # [nt, TS]
        selm = np.where(sel[:, :, None], t32, np.inf)
        LO[:, o] = selm.min(1)
        HI[:, o] = np.where(sel[:, :, None], t32, -np.inf).max(1)
        UBo[:, o] = np.where(sel, ubv.astype(np.float32), -np.inf).max(1)
    mask = np.zeros((nt, Y.shape[0]), bool)
    for o in range(8):
        db2 = (np.maximum(np.maximum(LO[:, o][:, None, :] - Y32[None, :, :],
                                     Y32[None, :, :] - HI[:, o][:, None, :]),
                          0.0) ** 2).sum(-1)
        mask |= db2 <= (UBo[:, o][:, None]) ** 2
    counts = np.maximum(mask.sum(1), 8)
    widths = np.ceil(counts / 64).astype(np.int64) * 64   # 64-mult per tile
    widths = np.minimum(widths, int(np.ceil(Y.shape[0] / 64) * 64))
    C = int(widths.max())
    ids = np.zeros((nt, C), np.int64)
    pad = np.zeros((nt, C), bool)
    for t in range(nt):
        ii = np.nonzero(mask[t])[0][:C]
        ids[t, :len(ii)] = ii
        pad[t, len(ii):] = True
    return ids, pad, widths


def _pack_y(Yg):
    """[W, 3] obj pts -> [13, W] bf16 rhs rows."""
    YT = Yg.T
    y2 = (YT ** 2).sum(0)
    yh, yl = _hi_lo(YT)
    y2h, y2l = _hi_lo(y2)
    out = np.zeros((13, Yg.shape[0]), ml_dtypes.bfloat16)
    out[0:3] = yh
    out[3:6] = yl
    out[6:9] = yh
    out[9] = y2h
    out[10] = y2l
    out[11] = 1.0
    out[12] = 1.0
    return out


def _pack_v(Vc):
    """[R, 3] verts -> [13, R] bf16 lhs rows; out = 2v.y - |y|^2 - |v|^2."""
    VT = Vc.T
    v2 = (VT ** 2).sum(0)
    vh, vl = _hi_lo(2.0 * VT)
    v2h, v2l = _hi_lo(v2)
    out = np.zeros((13, Vc.shape[0]), ml_dtypes.bfloat16)
    out[0:3] = vh
    out[3:6] = vh
    out[6:9] = vl
    out[9] = -1.0
    out[10] = -1.0
    out[11] = -v2h
    out[12] = -v2l
    return out


def _host_prep(verts, anchor_verts, obj_pts, contact_gaussians, K):
    V = np.asarray(verts[0], np.float64)
    Y = np.asarray(obj_pts[0], np.float64)
    A = np.asarray(anchor_verts[0], np.float64)
    cg = np.asarray(contact_gaussians, np.float64)
    N, P = V.shape[0], Y.shape[0]

    w_all = _host_weights(V, A, cg)               # [N] float32
    depth = int(np.log2(N // TS))
    pv = _kd_perm(V.astype(np.float32), depth)    # [N]
    Vs = V[pv]
    nt = N // TS
    assert nt % NCORES == 0
    tiles = Vs.reshape(nt, TS, 3)
    w2t = (w_all[pv] ** 2).astype(np.float32).reshape(nt, TS)
    ids, pad, widths = _candidates(tiles, Y, K)

    # snake-deal tiles (sorted by width desc) into NCORES x n_slots so
    # every core shares one slot-width profile (SPMD program shape);
    # interleave ranks fat/thin so drain-pairs (2s, 2s+1) are balanced
    n_slots = nt // NCORES
    order = np.argsort(-widths, kind='stable')
    ranks = np.empty(n_slots, np.int64)
    ranks[0::2] = np.arange(n_slots // 2)
    ranks[1::2] = n_slots - 1 - np.arange(n_slots - n_slots // 2)
    slot_w_sorted = widths[order].reshape(n_slots, NCORES).max(1).astype(int)
    slot_w = slot_w_sorted[ranks]
    Wtot = int(slot_w.sum())

    cores = []
    for c in range(NCORES):
        rhsb = np.zeros((13, Wtot), ml_dtypes.bfloat16)
        Vc = np.zeros((n_slots * TS, 3))
        w2c = np.ones((TS, n_slots + 1), np.float32)
        off = 0
        for s in range(n_slots):
            t = int(order[int(ranks[s]) * NCORES + c])
            Ws = int(slot_w[s])
            Yg = Y[ids[t, :Ws]].copy()
            Yg[pad[t, :Ws]] = 1.0e3
            rhsb[:, off:off + Ws] = _pack_y(Yg)
            Vc[s * TS:(s + 1) * TS] = tiles[t]
            w2c[:, s] = w2t[t]
            off += Ws
        # one operand tensor [13, NT*TS + Wtot]: verts block then candidates
        ops = np.concatenate([_pack_v(Vc), rhsb], axis=1)
        cores.append({
            "ops": np.ascontiguousarray(ops),
            "w2": np.ascontiguousarray(w2c),
        })
    return dict(cores=cores, slot_w=tuple(int(x) for x in slot_w),
                N=N, P=P, nt=nt)


def _pack_core(prep, core):
    return prep["cores"][core]


# ---------------------------------------------------------------------------
# Device program
# ---------------------------------------------------------------------------
def _build_kernel(slot_w=(1024,) * 16, K=5, n_cores=8):
    """One 128-vert tile per slot; slot s scans slot_w[s] candidate cols.
    Slots are drained from PSUM in pairs to halve Act instruction count."""
    NT = len(slot_w)
    Wtot = int(sum(slot_w))
    L = NT * TS                           # verts block width in ops tensor
    pmW = min(2048, max(512, max(slot_w)))
    bufs = max(1, min(4, 8 // ((pmW + 511) // 512)))

    # assign each slot a scan path to balance Act vs DVE busy time:
    # 'A' = Act drains PSUM->bf16, DVE pair-maxes (fast mode) + bf16 max8
    # 'D' = DVE max8 straight from PSUM (fp32 ranking, bf16 top-8 out)
    act_t, dve_t = 0.0, 0.0
    path = [None] * NT
    for t in sorted(range(NT), key=lambda i: slot_w[i]):
        W = slot_w[t]
        a_act = (W + 86) * 0.833 + 160
        a_dve = (W // 4 + W // 2 + 58) * 1.042
        d_dve = (W + 120) * 1.042
        if max(act_t + a_act, dve_t + a_dve) <= max(act_t, dve_t + d_dve):
            path[t] = 'A'
            act_t += a_act
            dve_t += a_dve
        else:
            path[t] = 'D'
            dve_t += d_dve
    nc = bass.Bass(num_devices=n_cores)

    ops_d = nc.dram_tensor("ops", [13, L + Wtot], BF16, kind="ExternalInput")
    w2_d = nc.dram_tensor("w2", [TS, NT + 1], F32, kind="ExternalInput")
    part_d = nc.dram_tensor("part", [1], F32, kind="ExternalOutput")

    offs = [L]
    for w in slot_w:
        offs.append(offs[-1] + w)

    with TileContext(nc) as tc:
        with tc.tile_pool(name="const", bufs=1) as cp:
            ops = cp.tile([13, L + Wtot], BF16, tag="ops")
            w2 = cp.tile([TS, NT + 1], F32, tag="w2")
            WK = cp.tile([TS, NT * 8], BF16, tag="WK")

            # preload the activation table first on the Act queue, then
            # HWDGE issues (SP + Act); first chunk = verts + slots 0-1
            warm = cp.tile([1, 2], F32, tag="warm")
            nc.gpsimd.memset(warm[:, 0:1], 0.0)
            nc.scalar.copy(warm[:, 1:2], warm[:, 0:1])
            cut1 = offs[min(2, NT)]
            cut2 = offs[min(8, NT)]
            nc.sync.dma_start(ops[:, 0:cut1], ops_d[:, 0:cut1])
            nc.scalar.dma_start(ops[:, cut1:cut2], ops_d[:, cut1:cut2])
            nc.sync.dma_start(ops[:, cut2:], ops_d[:, cut2:])
            nc.scalar.dma_start(w2[:], w2_d[:])

            tail_cuts = {NT // 2, NT - 2} if NT >= 8 else set()
            tail_prts = []
            tail_lo = [0]

            with tc.tile_pool(name="tail", bufs=1) as tl:
              with tc.tile_pool(name="psM", bufs=bufs, space="PSUM") as psM, \
                   tc.tile_pool(name="cand", bufs=3) as cnd:
                for t in range(NT):
                    W = slot_w[t]
                    base = offs[t]
                    h1 = W // 2
                    if W <= 2048:
                        pm = psM.tile([TS, pmW], F32, tag="pm")
                        for off in range(0, W, 512):
                            qw = min(512, W - off)
                            nc.tensor.matmul(
                                pm[:, off:off + qw],
                                ops[:, t * TS:(t + 1) * TS],
                                ops[:, base + off:base + off + qw])
                        if path[t] == 'A':
                            sb = cnd.tile([TS, W], BF16, tag=f"sb{W}")
                            nc.scalar.copy(sb[:], pm[:, 0:W])
                            mx = cnd.tile([TS, h1], BF16, tag=f"mx{W}")
                            nc.vector.tensor_tensor(mx[:], sb[:, 0:h1],
                                                    sb[:, h1:W],
                                                    op=mybir.AluOpType.max)
                            nc.vector.max(out=WK[:, t * 8:(t + 1) * 8],
                                          in_=mx[:])
                        else:
                            nc.vector.max(out=WK[:, t * 8:(t + 1) * 8],
                                          in_=pm[:, 0:W])
                    else:
                        nch = (W + 2047) // 2048
                        cands = cnd.tile([TS, nch * 8], F32, tag="cands")
                        for c in range(nch):
                            cw = min(2048, W - c * 2048)
                            pm = psM.tile([TS, pmW], F32, tag="pm")
                            for off in range(0, cw, 512):
                                qw = min(512, cw - off)
                                o2 = base + c * 2048 + off
                                nc.tensor.matmul(
                                    pm[:, off:off + qw],
                                    ops[:, t * TS:(t + 1) * TS],
                                    ops[:, o2:o2 + qw])
                            nc.vector.max(out=cands[:, c * 8:(c + 1) * 8],
                                          in_=pm[:, 0:cw])
                        t8 = cnd.tile([TS, 8], F32, tag="t8")
                        nc.vector.max(out=t8[:], in_=cands[:])
                        nc.vector.tensor_copy(WK[:, t * 8:(t + 1) * 8],
                                              t8[:])
                    if t + 1 in tail_cuts:
                        # mid-loop partial tail over completed slots
                        tail_prts.append(_emit_tail(nc, tl, WK, w2,
                                                    tail_lo[0], t + 1, NT, K,
                                                    f"p{t + 1}"))
                        tail_lo[0] = t + 1

              with tc.tile_pool(name="psT", bufs=1, space="PSUM") as psT:
                prtB = _emit_tail(nc, tl, WK, w2, tail_lo[0], NT, NT, K, "B")
                for p in tail_prts:
                    nc.vector.tensor_add(prtB[:], prtB[:], p[:])
                # cross-partition total via 1-col matmul against the
                # ones column -> single-descriptor output DMA
                tot = psT.tile([1, 1], F32, tag="tot")
                nc.tensor.matmul(tot[:], w2[:, NT:NT + 1], prtB[:])
                res = tl.tile([1, 1], F32, tag="res")
                nc.vector.tensor_copy(res[:], tot[:])
                nc.sync.dma_start(part_d[:], res[:, 0])
    return nc


def _emit_tail(nc, tl, WK, w2, lo, hi, NT, K, tag):
    """knn2 = max(-top8, 0) for slots [lo,hi); dot with w2; row partials."""
    n = hi - lo
    knn2 = tl.tile([TS, n * 8], F32, tag=f"knn2{tag}")
    nc.vector.tensor_scalar(knn2[:], WK[:, lo * 8:hi * 8], -1.0, 0.0,
                            op0=mybir.AluOpType.mult,
                            op1=mybir.AluOpType.max)
    wfin = tl.tile([TS, n * K], F32, tag=f"wfin{tag}")
    k3 = knn2[:].rearrange("p (t k) -> p t k", t=n, k=8)
    w3 = wfin[:].rearrange("p (t k) -> p t k", t=n, k=K)
    w2b = w2[:, lo:hi].unsqueeze(2).to_broadcast([TS, n, K])
    nc.vector.tensor_mul(w3, k3[:, :, :K], w2b)
    prt = tl.tile([TS, 1], F32, tag=f"prt{tag}")
    nc.vector.reduce_sum(prt[:], wfin[:], axis=mybir.AxisListType.X)
    return prt


_NC_CACHE = {}


def kernel(**inputs) -> np.ndarray:
    verts = np.asarray(inputs["verts"], np.float32)
    anchor_verts = np.asarray(inputs["anchor_verts"], np.float32)
    obj_pts = np.asarray(inputs["obj_pts"], np.float32)
    cg = np.asarray(inputs["contact_gaussians"], np.float32)
    K = int(np.asarray(inputs["K"]))
    B, N, _ = verts.shape
    P = obj_pts.shape[1]
    assert B == 1 and 1 <= K <= 8

    prep = _host_prep(verts, anchor_verts, obj_pts, cg, K)
    in_maps = [_pack_core(prep, c) for c in range(NCORES)]

    key = (prep["slot_w"], K)
    if key not in _NC_CACHE:
        _NC_CACHE[key] = _build_kernel(slot_w=prep["slot_w"], K=K,
                                       n_cores=NCORES)
    nc = _NC_CACHE[key]
    res = run_bass_kernel_spmd(nc, in_maps, core_ids=list(range(NCORES)))

    total = np.float32(0.0)
    for c in range(NCORES):
        total += np.float32(res.results[c]["part"].reshape(-1)[0])
    return np.float32(total / np.float32(N * K))
